# revision 46
# baseline (speedup 1.0000x reference)
"""HeteroMoE layer for Trainium2, 8-core SPMD.

Top-1 routing with weight 1.0: out[b] = expert_{argmax(logits[b])}(x[b]).
Host computes routing and permutes the batch into 8 cores x 4 slots with a
uniform compile-time mode per slot. All of stage 1 (depthwise 3x3 taps and
the expert-2 1x1 pre-matmul) runs as fp8e4m3 DoubleRow matmuls on the PE:
each DR matmul contracts two "terms", where a term is a 128x128 stationary
block (a tap diagonal, or an expert-2 weight block) applied to an [8x64]
window of the padded fp8 image at some plane/offset. The moving AP's
k-subtile dim strides by the offset difference between the paired terms.
The expert-2 matmul is error-compensated: M x ~= M8 x8 + M8 xr8 + MR8 x8
with xr8/MR8 the fp8-quantized residuals of x and M. BN is folded into the
conv weights; gelu+BN-bias on the scalar engine; the final pointwise 1x1
runs as fp8 DR for tap-only slots and fp16 for expert-2-carrying slots;
PSUM->SBUF copy+bias on DVE/Act; fp16 output.
"""
import numpy as np
import ml_dtypes

import concourse.bacc as bacc
import concourse.tile as tile
import concourse.mybir as mybir
from concourse.ap import AP
from concourse.bass_utils import run_bass_kernel_spmd

F32 = mybir.dt.float32
F16 = mybir.dt.float16
F8 = mybir.dt.float8e4
NPF8 = ml_dtypes.float8_e4m3
DRMODE = mybir.MatmulPerfMode.DoubleRow

B, C, H, W = 32, 256, 64, 64
HW = H * W
NCORES = 8
NSLOT = B // NCORES
PAD = 2                  # left/right pad cols
TPAD = 2                 # top pad rows
BPAD = 3                 # bottom pad rows (+1 so dummy terms stay in bounds)
R = W + 2 * PAD          # padded row stride (68)
RH = H + TPAD + BPAD     # padded rows (69)
PADHW = R * RH           # 4692
NBLK = C // 128          # 2 channel blocks
EPS = 1e-5

_CACHE = {}


def _offsets(dil):
    return [(dy * dil, dx * dil) for dy in (-1, 0, 1) for dx in (-1, 0, 1)]


def _slot_offsets(mode):
    if mode == "d1":
        return _offsets(1)
    if mode == "d2":
        return _offsets(2)
    if mode == "d12":
        return _offsets(1) + [o for o in _offsets(2) if o != (0, 0)]
    return []


def _slot_terms(mode, has_m):
    """Stage-1 contraction term list (shared by device code and host packer).

    Returns (terms, pairs, nplanes):
      terms: list of (kind, rhs_offset) where kind is ("tap", o1d) or
             ("m8", ib) / ("m8x", ib) / ("mr8", ib); rhs_offset is relative to
             the per-chunk window base, with the cb_out tap plane selected at
             emit time via +cb*PADHW for taps only.
      pairs: list of (i, j|None) indices into terms (None = zero dummy).
      nplanes: image planes in the slot's x tile (2 or 4).
    """
    offs = sorted(dy * R + dx for (dy, dx) in _slot_offsets(mode))
    terms = [(("tap", o), o) for o in offs]
    nplanes = NBLK
    if has_m:
        nplanes = 2 * NBLK
        for ib in range(NBLK):
            terms.append((("m8", ib), ib * PADHW))
        for ib in range(NBLK):
            terms.append((("m8x", ib), (NBLK + ib) * PADHW))
        for ib in range(NBLK):
            terms.append((("mr8", ib), ib * PADHW))
    pairs = []
    i = 0
    while i < len(terms):
        if i + 1 < len(terms):
            pairs.append((i, i + 1))
            i += 2
        else:
            pairs.append((i, None))
            i += 1
    return terms, pairs, nplanes


def build(slot_modes, repeat=1):
    """slot_modes: (tapmode, has_m) per slot."""
    nc = bacc.Bacc("TRN2", target_bir_lowering=False, debug=False,
                   num_devices=NCORES)
    sinfo = {}
    prm = {}
    xin = {}
    for s, (tm, has_m) in enumerate(slot_modes):
        terms, pairs, nplanes = _slot_terms(tm, has_m)
        sinfo[s] = (terms, pairs, nplanes)
        if not terms:
            continue
        xin[s] = nc.dram_tensor(f"xin{s}", [nplanes, 128, PADHW], F8,
                                kind="ExternalInput").ap()
        prm[f"dk{s}"] = nc.dram_tensor(
            f"dk{s}", [128, NBLK * len(pairs) * 2 * 128], F8,
            kind="ExternalInput").ap()
        prm[f"pw{s}"] = nc.dram_tensor(
            f"pw{s}", [128, NBLK * NBLK * 128], F16 if has_m else F8,
            kind="ExternalInput").ap()
        prm[f"tb{s}"] = nc.dram_tensor(f"tb{s}", [128, NBLK], F32,
                                       kind="ExternalInput").ap()
        prm[f"qb{s}"] = nc.dram_tensor(f"qb{s}", [128, NBLK], F32,
                                       kind="ExternalInput").ap()
    yout = nc.dram_tensor("yout", [NSLOT, C, HW], F16, kind="ExternalOutput").ap()
    live = [s for s in range(NSLOT) if sinfo[s][0]]
    n2 = sum(1 for s in live if sinfo[s][2] == NBLK)
    n4 = len(live) - n2

    with tile.TileContext(nc) as tc:
        with tc.tile_pool(name="params", bufs=1) as ppool, \
             tc.tile_pool(name="x8a", bufs=max(1, n2)) as xpool2, \
             tc.tile_pool(name="x8b", bufs=max(1, n4)) as xpool4, \
             tc.tile_pool(name="a16", bufs=3) as apool, \
             tc.tile_pool(name="o16", bufs=4) as opool, \
             tc.tile_pool(name="psz", bufs=2, space="PSUM") as pszp, \
             tc.tile_pool(name="psw", bufs=2, space="PSUM") as pswp:

            pt = {}

            def load_param(name):
                ap = prm[name]
                t = ppool.tile(list(ap.shape), ap.dtype, tag=name, name=name)
                nc.sync.dma_start(t[:], ap)
                pt[name] = t

            # PE warm-up: dummy matmul chain keeps the tensor engine busy
            # while the first DMAs land, so the p-state ramp completes
            # before real work starts.
            wtile = ppool.tile([128, 128], F8, tag="warm", name="warm")
            nc.gpsimd.memset(wtile[:], 0)
            wps = pszp.tile([128, 1024], F32, tag="psz", name="warmps")
            for _ in range(26):
                nc.tensor.matmul(wps[:, 0:128], wtile[:], wtile[:],
                                 start=True, stop=True)

            for rep in range(repeat):
                x8 = {}
                first = True
                for s in live:
                    nplanes = sinfo[s][2]
                    pool = xpool2 if nplanes == NBLK else xpool4
                    t = pool.tile([128, nplanes, PADHW], F8,
                                  tag=f"x8_{nplanes}", name=f"x8{s}")
                    src = xin[s].rearrange("a p w -> p a w")
                    if first:
                        # fine-grained first upload so the PE starts early
                        htop = (TPAD + 2 * 8) * R
                        nc.sync.dma_start(t[:, 0, 0:htop], src[:, 0, 0:htop])
                        if rep == 0:
                            load_param(f"dk{s}")
                            load_param(f"tb{s}")
                        nc.sync.dma_start(t[:, 0, htop:PADHW],
                                          src[:, 0, htop:PADHW])
                        for pl in range(1, nplanes):
                            nc.sync.dma_start(t[:, pl], src[:, pl])
                        first = False
                    else:
                        if rep == 0:
                            load_param(f"dk{s}")
                            load_param(f"tb{s}")
                        for pl in range(nplanes):
                            nc.sync.dma_start(t[:, pl], src[:, pl])
                    x8[s] = t
                if rep == 0:
                    for s in live:
                        load_param(f"pw{s}")
                        load_param(f"qb{s}")

                a16 = {}

                def stage1(s, alt_pool=False):
                    tm, has_m = slot_modes[s]
                    terms, pairs, nplanes = sinfo[s]
                    xt = x8[s][:]
                    xoff = xt.offset
                    pstride = nplanes * PADHW
                    at = apool.tile([128, NBLK, HW], F16 if has_m else F8,
                                    tag="a16m" if has_m else "a8",
                                    name=f"a16_{s}")
                    a16[s] = at
                    dk3 = pt[f"dk{s}"][:].rearrange(
                        "p (c i j m) -> p c i j m",
                        c=NBLK, i=len(pairs), j=2, m=128)

                    def term_off(k, cb):
                        kind, off = terms[k]
                        if kind[0] == "tap":
                            return cb * PADHW + off
                        return off

                    for cb in range(NBLK):
                        for hf4 in range(4):
                            if alt_pool and (hf4 + cb) % 2 == 1:
                                psz = pswp.tile([128, 1024], F32, tag="psw",
                                                name="psz")
                            else:
                                psz = pszp.tile([128, 1024], F32, tag="psz",
                                                name="psz")
                            for q in range(2):
                                chunk = hf4 * 2 + q
                                wbase = (TPAD + chunk * 8) * R + PAD
                                pslice = psz[:, q * 512:(q + 1) * 512]
                                for i, (ta, tb_) in enumerate(pairs):
                                    o1 = term_off(ta, cb)
                                    o2 = (o1 + 1 if tb_ is None
                                          else term_off(tb_, cb))
                                    rhs = AP(xt.tensor, xoff + wbase + o1,
                                             [[pstride, 128], [o2 - o1, 2],
                                              [R, 8], [1, W]])
                                    nc.tensor.matmul(
                                        pslice, dk3[:, cb, i], rhs,
                                        start=(i == 0),
                                        stop=(i == len(pairs) - 1),
                                        perf_mode=DRMODE)
                            nc.scalar.activation(
                                at[:, cb, hf4 * 1024:(hf4 + 1) * 1024],
                                psz[:],
                                mybir.ActivationFunctionType.Gelu,
                                bias=pt[f"tb{s}"][:, cb:cb + 1], scale=1.0)

                def stage2(s, alt_pool=False, fine_out=False, tail=False):
                    has_m = slot_modes[s][1]
                    at = a16[s][:]
                    aoff = at.offset
                    pw3 = pt[f"pw{s}"][:].rearrange(
                        "p (c i m) -> p c i m", c=NBLK, i=NBLK, m=128)
                    for cb in range(NBLK):
                        o16 = opool.tile([128, HW], F16, tag="o16",
                                         name=f"o16_{s}_{cb}")
                        for quad in range(4):
                            if alt_pool and (quad + cb) % 2 == 1:
                                psw = pszp.tile([128, 1024], F32, tag="psz",
                                                name="psw")
                            else:
                                psw = pswp.tile([128, 1024], F32, tag="psw",
                                                name="psw")
                            for q in range(2):
                                off = (quad * 2 + q) * 512
                                if has_m:
                                    for ib in range(NBLK):
                                        nc.tensor.matmul(
                                            psw[:, q * 512:(q + 1) * 512],
                                            pw3[:, cb, ib],
                                            a16[s][:, ib, off:off + 512],
                                            start=(ib == 0),
                                            stop=(ib == NBLK - 1))
                                else:
                                    rhs = AP(at.tensor, aoff + off,
                                             [[NBLK * HW, 128], [HW, 2],
                                              [1, 512]])
                                    nc.tensor.matmul(
                                        psw[:, q * 512:(q + 1) * 512],
                                        pw3[:, cb], rhs,
                                        start=True, stop=True,
                                        perf_mode=DRMODE)
                            qbcol = pt[f"qb{s}"][:, cb:cb + 1]
                            if tail and quad == 3:
                                # last quad: 512-halves on both engines in
                                # parallel for the shortest tail
                                for q in range(2):
                                    dst = o16[:, quad * 1024 + q * 512:
                                              quad * 1024 + (q + 1) * 512]
                                    srcp = psw[:, q * 512:(q + 1) * 512]
                                    if q == 0:
                                        nc.vector.tensor_scalar_add(
                                            dst, srcp, qbcol)
                                    else:
                                        nc.scalar.activation(
                                            dst, srcp,
                                            mybir.ActivationFunctionType.Identity,
                                            bias=qbcol, scale=1.0)
                                continue_copy = True
                            else:
                                dst = o16[:, quad * 1024:(quad + 1) * 1024]
                                if tail:
                                    on_dve = (quad + cb) % 2 == 0
                                else:
                                    on_dve = quad != 2  # 3 of 4 on DVE
                                if on_dve:
                                    nc.vector.tensor_scalar_add(
                                        dst, psw[:], qbcol)
                                else:
                                    nc.scalar.activation(
                                        dst, psw[:],
                                        mybir.ActivationFunctionType.Identity,
                                        bias=qbcol, scale=1.0)
                        nout = 4 if fine_out else 2
                        for hh in range(nout):
                            w0 = HW // nout
                            nc.sync.dma_start(
                                yout[s, cb * 128:(cb + 1) * 128,
                                     hh * w0:(hh + 1) * w0],
                                o16[:, hh * w0:(hh + 1) * w0])

                # software pipeline: keep PE busy during gelu of prior slot;
                # the tail stage-2 borrows the idle psz banks for deeper
                # PSUM buffering and streams output at finer grain.
                # Order cheap-pointwise slots early, fp16-pointwise last so
                # its PE work overlaps its own copies.
                order = list(live)
                st2 = sorted(order, key=lambda s: slot_modes[s][1])
                emitted = set()
                k2 = 0
                for j, s in enumerate(order):
                    stage1(s, alt_pool=(j < 2))
                    if j >= 1 and k2 < len(st2) and st2[k2] in a16:
                        stage2(st2[k2]); emitted.add(st2[k2]); k2 += 1
                rest = [s for s in st2 if s not in emitted]
                for j, s in enumerate(rest):
                    last = j == len(rest) - 1
                    stage2(s, alt_pool=(j >= len(rest) - 2),
                           fine_out=(j >= len(rest) - 2), tail=last)
    nc.compile()
    return nc


def _plan(idx):
    """Assign elements to (core, slot); return slot_modes, elem[core][slot]."""
    by = [list(np.where(idx == t)[0]) for t in range(3)]
    groups = []  # [mode, has_m, elems]
    for t, mode in ((0, "d1"), (1, "d2")):
        while len(by[t]) >= 8:
            groups.append([mode, False, by[t][:8]])
            by[t] = by[t][8:]
    rem1, rem2 = by[0], by[1]
    if rem1 or rem2:
        if len(rem1) + len(rem2) <= 8:
            g = rem1 + rem2
            mode = "d12" if (rem1 and rem2) else ("d1" if rem1 else "d2")
            take = min(8 - len(g), len(by[2]))
            g = g + by[2][:take]
            by[2] = by[2][take:]
            groups.append([mode, take > 0, g])
        else:
            for t, mode in ((0, "d1"), (1, "d2")):
                if by[t]:
                    take = min(8 - len(by[t]), len(by[2]))
                    g = by[t] + by[2][:take]
                    by[2] = by[2][take:]
                    groups.append([mode, take > 0, g])
        by[0] = by[1] = []
    while by[2]:
        groups.append([None, True, by[2][:8]])
        by[2] = by[2][8:]
    while len(groups) > NSLOT:
        tapg = [g for g in groups if g[0] is not None]
        a, b = tapg[-2], tapg[-1]
        groups.remove(b)
        a[0] = "d12"
        a[1] = a[1] or b[1]
        a[2] += b[2]
        assert len(a[2]) <= 8
    for g in groups:
        while len(g[2]) < 8:
            g[2].append(-1)
    while len(groups) < NSLOT:
        groups.append([None, False, [-1] * 8])
    slot_modes = tuple((g[0], g[1]) for g in groups)
    elem = [[groups[s][2][c] for s in range(NSLOT)] for c in range(NCORES)]
    return slot_modes, elem


def _fold_params(kw):
    out = {}
    for i in range(3):
        g = kw[f"e{i}_g"]; b = kw[f"e{i}_b"]; m = kw[f"e{i}_m"]; v = kw[f"e{i}_v"]
        s = g / np.sqrt(v + EPS)
        t = b - m * s
        out[i] = dict(s=s.astype(np.float32), t=t.astype(np.float32),
                      pw=kw[f"e{i}_pw"].astype(np.float32),
                      pb=kw[f"e{i}_pb"].astype(np.float32))
        if i < 2:
            out[i]["k"] = (kw[f"e{i}_k"].reshape(C, 9) * s[:, None]).astype(np.float32)
        else:
            out[i]["M"] = (kw["e2_k"] * s[:, None]).astype(np.float32)
    return out


def _make_inmaps(x, idx, elem, slot_modes, fold):
    x8full = np.zeros((B, C, RH, R), NPF8)
    x8full[:, :, TPAD:TPAD + H, PAD:PAD + W] = x.astype(NPF8)
    x8full = x8full.reshape(B, C, PADHW)
    xr8full = np.zeros((B, C, RH, R), NPF8)
    xr8full[:, :, TPAD:TPAD + H, PAD:PAD + W] = \
        (x - x8full.reshape(B, C, RH, R)[:, :, TPAD:TPAD + H,
                                         PAD:PAD + W].astype(np.float32)
         ).astype(NPF8)
    xr8full = xr8full.reshape(B, C, PADHW)

    in_maps = []
    rng = np.arange(128)
    for c in range(NCORES):
        im = {}
        for s, (tm, has_m) in enumerate(slot_modes):
            terms, pairs, nplanes = _slot_terms(tm, has_m)
            if not terms:
                continue
            e = elem[c][s]
            t_e = idx[e] if e >= 0 else -1
            f = fold[t_e] if t_e >= 0 else None

            xs = np.zeros((nplanes, 128, PADHW), NPF8)
            if e >= 0:
                for bk in range(NBLK):
                    xs[bk] = x8full[e, bk * 128:(bk + 1) * 128]
                if has_m:
                    for bk in range(NBLK):
                        xs[NBLK + bk] = xr8full[e, bk * 128:(bk + 1) * 128]
            im[f"xin{s}"] = xs

            dk = np.zeros((128, NBLK, len(pairs), 2, 128), np.float32)
            if f is not None:
                if t_e < 2:
                    myoffs = [dy * R + dx for (dy, dx) in
                              _offsets(1 if t_e == 0 else 2)]
                    for i, pr in enumerate(pairs):
                        for j, k_ in enumerate(pr):
                            if k_ is None:
                                continue
                            kind, _ = terms[k_]
                            if kind[0] == "tap" and kind[1] in myoffs:
                                ki = myoffs.index(kind[1])
                                for bk in range(NBLK):
                                    dk[rng, bk, i, j, rng] = \
                                        f["k"][bk * 128:(bk + 1) * 128, ki]
                else:
                    M = f["M"]
                    M8 = M.astype(NPF8)
                    MR = (M - M8.astype(np.float32)).astype(NPF8)
                    M8f = M8.astype(np.float32)
                    MRf = MR.astype(np.float32)
                    for i, pr in enumerate(pairs):
                        for j, k_ in enumerate(pr):
                            if k_ is None:
                                continue
                            kind, _ = terms[k_]
                            if kind[0] in ("m8", "m8x", "mr8"):
                                src = MRf if kind[0] == "mr8" else M8f
                                ib = kind[1]
                                for cb in range(NBLK):
                                    dk[:, cb, i, j, :] = \
                                        src[cb * 128:(cb + 1) * 128,
                                            ib * 128:(ib + 1) * 128].T
            im[f"dk{s}"] = dk.astype(NPF8).reshape(128, -1)

            pw = np.zeros((128, NBLK, NBLK, 128), np.float32)
            tb = np.zeros((128, NBLK), np.float32)
            qb = np.zeros((128, NBLK), np.float32)
            if f is not None:
                P = f["pw"]
                for cb in range(NBLK):
                    for ib in range(NBLK):
                        pw[:, cb, ib, :] = \
                            P[cb * 128:(cb + 1) * 128,
                              ib * 128:(ib + 1) * 128].T
                tb[:] = f["t"].reshape(NBLK, 128).T
                qb[:] = f["pb"].reshape(NBLK, 128).T
            im[f"pw{s}"] = pw.astype(
                np.float16 if has_m else NPF8).reshape(128, -1)
            im[f"tb{s}"] = tb
            im[f"qb{s}"] = qb
        in_maps.append(im)
    return in_maps


def kernel(**inputs):
    inputs = {k: np.asarray(v) for k, v in inputs.items()}
    x = np.ascontiguousarray(inputs["x"], np.float32)
    rw = np.asarray(inputs["rw"], np.float32)
    rb = np.asarray(inputs["rb"], np.float32)
    pooled = x.mean(axis=(2, 3), dtype=np.float32)
    logits = pooled @ rw.T + rb
    idx = logits.argmax(-1)

    slot_modes, elem = _plan(idx)
    fold = _fold_params(inputs)
    in_maps = _make_inmaps(x, idx, elem, slot_modes, fold)

    if slot_modes not in _CACHE:
        _CACHE[slot_modes] = build(slot_modes)
    nc = _CACHE[slot_modes]
    res = run_bass_kernel_spmd(nc, in_maps, core_ids=list(range(NCORES)),
                               trace=False)
    out = np.zeros((B, C, H, W), np.float32)
    for c in range(NCORES):
        yo = res.results[c]["yout"]
        for s in range(NSLOT):
            e = elem[c][s]
            if e >= 0:
                out[e] = yo[s].astype(np.float32).reshape(C, H, W)
    return out
